# revision 2
# baseline (speedup 1.0000x reference)
"""Multi-head attention block on 8 TRN2 NeuronCores.

Sharding: core c -> (batch b = c//2, head-group hg = c%2).
Each core computes QKV projections for its 8 heads over its batch
(fp32r matmuls), attention (fp32r QK^T, exp on ACT, bf16 A@V with a
col-packed ones-matmul producing replicated row-sums), and a bf16
output projection of its head-group's channels. Pairs of cores
(same batch) combine partial projections with per-q-block
ReduceScatter collectives; the host concatenates the 8 per-core
output shards into the full [4, 2048, 1024] result.
"""

import sys

if "/opt/trn_rl_repo" not in sys.path:
    sys.path.insert(0, "/opt/trn_rl_repo")

import numpy as np
import ml_dtypes

N_CORES = 8
B, T, DIM = 4, 2048, 1024
H_TOT, HD = 16, 64
HPC = H_TOT // 2          # heads per core (2 head-groups)
DQ = HPC * HD             # 512: per-core q/k/v width
SCALE = HD ** -0.5
CH = 2                    # k-blocks per exp chunk
KB_T = T // 128           # 16 k-blocks over sequence
KB_C = DIM // 128         # 8 k-blocks over channel dim

_CACHE = {}


def _build():
    import concourse.bass as bass
    import concourse.tile as tile
    from concourse import bacc, mybir

    F32 = mybir.dt.float32
    F32R = mybir.dt.float32r
    BF16 = mybir.dt.bfloat16
    AF = mybir.ActivationFunctionType

    nc = bacc.Bacc("TRN2", target_bir_lowering=False, debug=False,
                   num_devices=N_CORES)

    x_t = nc.dram_tensor("x_t", [DIM, T], F32, kind="ExternalInput").ap()
    w_qkv = nc.dram_tensor("w_qkv_s", [DIM, 3 * DQ], F32, kind="ExternalInput").ap()
    b_qkv = nc.dram_tensor("b_qkv_s", [3 * DQ], F32, kind="ExternalInput").ap()
    w_proj = nc.dram_tensor("w_proj_s", [DQ, DIM], BF16, kind="ExternalInput").ap()
    b_proj = nc.dram_tensor("b_proj_h", [DIM], F32, kind="ExternalInput").ap()
    out = nc.dram_tensor("out", [T // 2, DIM], F32, kind="ExternalOutput").ap()
    partial = nc.dram_tensor("partial", [T, DIM], F32).ap()
    rs_out = nc.dram_tensor("rs_out", [T // 2, DIM], F32).ap()

    groups = [[0, 1], [2, 3], [4, 5], [6, 7]]

    def bcast_ap(src_ap, parts):
        # partition-broadcast read of a 1-D DRAM row
        return bass.AP(tensor=src_ap.tensor, offset=src_ap.offset,
                       ap=[[0, parts]] + list(src_ap.ap))

    with tile.TileContext(nc) as tc:
        with (
            tc.tile_pool(name="persist", bufs=1) as pp,
        ):
            q_sb = pp.tile([128, 4, T], F32R)
            k_sb = pp.tile([128, 4, T], F32R)
            v_sb = pp.tile([128, KB_T, DQ], BF16)
            ones_bf = pp.tile([128, HD], BF16)
            bqkv_sb = pp.tile([128, 12], F32)
            bv_sb = pp.tile([128, DQ], F32)

            nc.vector.memset(ones_bf[:], 1.0)
            nc.sync.dma_start(out=bqkv_sb, in_=b_qkv.rearrange("(m p) -> p m", p=128))
            nc.sync.dma_start(out=bv_sb, in_=bcast_ap(b_qkv[2 * DQ:3 * DQ], 128))

            # ---------------- Phase A: QKV projections ----------------
            with (
                tc.tile_pool(name="phA", bufs=1) as pa,
                tc.tile_pool(name="wch", bufs=2) as wch,
                tc.tile_pool(name="psA", bufs=2, space="PSUM") as psa,
                tc.tile_pool(name="psV", bufs=2, space="PSUM") as psv,
            ):
                x_sb = pa.tile([128, KB_C, T], F32R)
                for kb in range(KB_C):
                    nc.sync.dma_start(
                        out=x_sb[:, kb, :],
                        in_=x_t[128 * kb:128 * (kb + 1), :].bitcast(F32R))

                # Q then K: out^T layout [dq, t], stationary w block, moving x^T
                for which, dst in ((0, q_sb), (1, k_sb)):
                    w_c = wch.tile([128, KB_C, DQ], F32R, tag="wchunk")
                    for kb in range(KB_C):
                        nc.sync.dma_start(
                            out=w_c[:, kb, :],
                            in_=w_qkv[128 * kb:128 * (kb + 1),
                                      DQ * which:DQ * (which + 1)].bitcast(F32R))
                    for m in range(4):
                        for nbh in range(2):
                            ps = psa.tile([128, 1024], F32)
                            for kb in range(KB_C):
                                for nb2 in range(2):
                                    tcol = 1024 * nbh + 512 * nb2
                                    nc.tensor.matmul(
                                        ps[:, 512 * nb2:512 * (nb2 + 1)],
                                        w_c[:, kb, 128 * m:128 * (m + 1)],
                                        x_sb[:, kb, tcol:tcol + 512],
                                        start=(kb == 0), stop=(kb == KB_C - 1))
                            nc.scalar.activation(
                                out=dst[:, m, 1024 * nbh:1024 * (nbh + 1)],
                                in_=ps[:],
                                func=AF.Identity,
                                bias=bqkv_sb[:, 4 * which + m:4 * which + m + 1],
                                scale=1.0)

                # V: natural [t, dv] layout; stationary x^T block, moving w_v
                w_c = wch.tile([128, KB_C, DQ], F32R, tag="wchunk")
                for kb in range(KB_C):
                    nc.sync.dma_start(
                        out=w_c[:, kb, :],
                        in_=w_qkv[128 * kb:128 * (kb + 1),
                                  2 * DQ:3 * DQ].bitcast(F32R))
                for tb in range(KB_T):
                    ps = psv.tile([128, DQ], F32)
                    for kb in range(KB_C):
                        nc.tensor.matmul(
                            ps[:],
                            x_sb[:, kb, 128 * tb:128 * (tb + 1)],
                            w_c[:, kb, :],
                            start=(kb == 0), stop=(kb == KB_C - 1))
                    nc.vector.tensor_add(v_sb[:, tb, :], ps[:], bv_sb[:])

            # ---------------- Phase B: attention + proj + RS ----------------
            with (
                tc.tile_pool(name="zb", bufs=1) as zb,
                tc.tile_pool(name="apool", bufs=3) as apool,
                tc.tile_pool(name="small", bufs=4) as small,
                tc.tile_pool(name="opool", bufs=3) as opool,
                tc.tile_pool(name="psS", bufs=3, space="PSUM") as pss,
                tc.tile_pool(name="psZ", bufs=2, space="PSUM") as psz,
            ):
                z_sb = zb.tile([128, 4, T], BF16)
                wp_sb = zb.tile([128, 4, DIM], BF16)
                bp_sb = zb.tile([128, DIM], F32)
                nc.sync.dma_start(
                    out=wp_sb, in_=w_proj.rearrange("(m p) c -> p m c", p=128))
                nc.sync.dma_start(out=bp_sb, in_=bcast_ap(b_proj[:], 128))

                for qb in range(4):
                    q0 = 512 * qb
                    for hp in range(4):
                        z0 = psz.tile([128, 512], F32, tag="z")
                        z1 = psz.tile([128, 512], F32, tag="z")
                        for ch in range(KB_T // CH):
                            s0 = pss.tile([128, CH, 512], F32, tag="s")
                            s1 = pss.tile([128, CH, 512], F32, tag="s")
                            for i in range(CH):
                                kb = CH * ch + i
                                kc = 128 * kb
                                nc.tensor.matmul(
                                    s0[:, i, :],
                                    k_sb[0:64, hp, kc:kc + 128],
                                    q_sb[0:64, hp, q0:q0 + 512],
                                    start=True, stop=True)
                                nc.tensor.matmul(
                                    s1[:, i, :],
                                    k_sb[64:128, hp, kc:kc + 128],
                                    q_sb[64:128, hp, q0:q0 + 512],
                                    start=True, stop=True)
                            a0 = apool.tile([128, CH, 512], BF16, tag="a")
                            a1 = apool.tile([128, CH, 512], BF16, tag="a")
                            nc.scalar.activation(out=a0[:], in_=s0[:],
                                                 func=AF.Exp, scale=SCALE)
                            nc.scalar.activation(out=a1[:], in_=s1[:],
                                                 func=AF.Exp, scale=SCALE)
                            for i in range(CH):
                                kb = CH * ch + i
                                st = (kb == 0)
                                sp = (kb == KB_T - 1)
                                h0c = 64 * (2 * hp)
                                h1c = 64 * (2 * hp + 1)
                                nc.tensor.matmul(
                                    z0[0:64, :], v_sb[:, kb, h0c:h0c + 64],
                                    a0[:, i, :], start=st, stop=sp,
                                    tile_position=(0, 0))
                                nc.tensor.matmul(
                                    z0[64:128, :], ones_bf[:],
                                    a0[:, i, :], start=st, stop=sp,
                                    tile_position=(0, 64))
                                nc.tensor.matmul(
                                    z1[0:64, :], v_sb[:, kb, h1c:h1c + 64],
                                    a1[:, i, :], start=st, stop=sp,
                                    tile_position=(0, 0))
                                nc.tensor.matmul(
                                    z1[64:128, :], ones_bf[:],
                                    a1[:, i, :], start=st, stop=sp,
                                    tile_position=(0, 64))
                        for hh, zz in ((0, z0), (1, z1)):
                            rinv = small.tile([64, 512], F32, tag="rinv")
                            nc.vector.reciprocal(rinv[:], zz[64:128, :])
                            nc.vector.tensor_mul(
                                z_sb[64 * hh:64 * hh + 64, hp, q0:q0 + 512],
                                zz[0:64, :], rinv[:])

                    # output projection for this q-block
                    for tb4 in range(4):
                        t0 = q0 + 128 * tb4
                        for cb in range(2):
                            ppj = psz.tile([128, 512], F32, tag="z")
                            for m in range(4):
                                nc.tensor.matmul(
                                    ppj[:],
                                    z_sb[:, m, t0:t0 + 128],
                                    wp_sb[:, m, 512 * cb:512 * (cb + 1)],
                                    start=(m == 0), stop=(m == 3))
                            o = opool.tile([128, 512], F32, tag="o")
                            nc.vector.tensor_add(
                                o[:], ppj[:], bp_sb[:, 512 * cb:512 * (cb + 1)])
                            nc.sync.dma_start(
                                out=partial[t0:t0 + 128, 512 * cb:512 * (cb + 1)],
                                in_=o[:])

                    # pairwise reduce-scatter of this q-block, then store
                    nc.gpsimd.collective_compute(
                        "ReduceScatter",
                        mybir.AluOpType.add,
                        ins=[partial[q0:q0 + 512, :]],
                        outs=[rs_out[256 * qb:256 * (qb + 1), :]],
                        replica_groups=groups,
                    )
                    nc.sync.dma_start(
                        out=out[256 * qb:256 * (qb + 1), :],
                        in_=rs_out[256 * qb:256 * (qb + 1), :])

    nc.compile()
    return nc


def _get_nc():
    if "nc" not in _CACHE:
        _CACHE["nc"] = _build()
    return _CACHE["nc"]


def kernel(x, w_qkv, b_qkv, w_proj, b_proj):
    from concourse.bass_utils import run_bass_kernel_spmd

    x = np.asarray(x, dtype=np.float32)
    w_qkv = np.asarray(w_qkv, dtype=np.float32)
    b_qkv = np.asarray(b_qkv, dtype=np.float32)
    w_proj = np.asarray(w_proj, dtype=np.float32)
    b_proj = np.asarray(b_proj, dtype=np.float32)

    nc = _get_nc()

    in_maps = []
    for c in range(N_CORES):
        b = c // 2
        hg = c % 2
        cols = slice(DQ * hg, DQ * (hg + 1))
        w_s = np.ascontiguousarray(np.concatenate(
            [w_qkv[:, 0:DIM][:, cols],
             w_qkv[:, DIM:2 * DIM][:, cols],
             w_qkv[:, 2 * DIM:3 * DIM][:, cols]], axis=1))
        b_s = np.ascontiguousarray(np.concatenate(
            [b_qkv[0:DIM][cols], b_qkv[DIM:2 * DIM][cols],
             b_qkv[2 * DIM:3 * DIM][cols]]))
        in_maps.append({
            "x_t": np.ascontiguousarray(x[b].T),
            "w_qkv_s": w_s,
            "b_qkv_s": b_s,
            "w_proj_s": np.ascontiguousarray(
                w_proj[DQ * hg:DQ * (hg + 1), :]).astype(ml_dtypes.bfloat16),
            "b_proj_h": (b_proj * 0.5).astype(np.float32),
        })

    res = run_bass_kernel_spmd(nc, in_maps, core_ids=list(range(N_CORES)))

    full = np.empty((B, T, DIM), dtype=np.float32)
    for c in range(N_CORES):
        b = c // 2
        p = c % 2
        o = res.results[c]["out"]
        for qb in range(4):
            full[b, 512 * qb + 256 * p:512 * qb + 256 * (p + 1), :] = \
                o[256 * qb:256 * (qb + 1), :]
    return full


# revision 3
# speedup vs baseline: 1.0435x; 1.0435x over previous
"""Multi-head attention block on 8 TRN2 NeuronCores.

Sharding: core c -> (batch b = c//2, head-group hg = c%2).
Each core computes QKV projections for its 8 heads over its batch
(fp32r matmuls), attention (fp32r QK^T, exp on ACT, bf16 A@V with a
col-packed ones-matmul producing replicated row-sums), and a bf16
output projection of its head-group's channels. Pairs of cores
(same batch) combine partial projections with per-q-block
ReduceScatter collectives; the host concatenates the 8 per-core
output shards into the full [4, 2048, 1024] result.
"""

import sys

if "/opt/trn_rl_repo" not in sys.path:
    sys.path.insert(0, "/opt/trn_rl_repo")

import numpy as np
import ml_dtypes

N_CORES = 8
B, T, DIM = 4, 2048, 1024
H_TOT, HD = 16, 64
HPC = H_TOT // 2          # heads per core (2 head-groups)
DQ = HPC * HD             # 512: per-core q/k/v width
SCALE = HD ** -0.5
CH = 2                    # k-blocks per exp chunk
KB_T = T // 128           # 16 k-blocks over sequence
KB_C = DIM // 128         # 8 k-blocks over channel dim

_CACHE = {}


def _build():
    import concourse.bass as bass
    import concourse.tile as tile
    from concourse import bacc, mybir

    F32 = mybir.dt.float32
    F32R = mybir.dt.float32r
    BF16 = mybir.dt.bfloat16
    AF = mybir.ActivationFunctionType

    nc = bacc.Bacc("TRN2", target_bir_lowering=False, debug=False,
                   num_devices=N_CORES)

    x_t = nc.dram_tensor("x_t", [DIM, T], F32, kind="ExternalInput").ap()
    w_qkv = nc.dram_tensor("w_qkv_s", [DIM, 3 * DQ], F32, kind="ExternalInput").ap()
    b_qkv = nc.dram_tensor("b_qkv_s", [3 * DQ], F32, kind="ExternalInput").ap()
    w_proj = nc.dram_tensor("w_proj_s", [DQ, DIM], BF16, kind="ExternalInput").ap()
    b_proj = nc.dram_tensor("b_proj_h", [DIM], F32, kind="ExternalInput").ap()
    out = nc.dram_tensor("out", [T // 2, DIM], F32, kind="ExternalOutput").ap()
    partial = nc.dram_tensor("partial", [T, DIM], F32).ap()
    rs_out = nc.dram_tensor("rs_out", [T // 2, DIM], F32).ap()

    groups = [[0, 1], [2, 3], [4, 5], [6, 7]]

    def bcast_ap(src_ap, parts):
        # partition-broadcast read of a 1-D DRAM row
        return bass.AP(tensor=src_ap.tensor, offset=src_ap.offset,
                       ap=[[0, parts]] + list(src_ap.ap))

    with tile.TileContext(nc) as tc:
        with (
            tc.tile_pool(name="persist", bufs=1) as pp,
        ):
            q_sb = pp.tile([128, 4, T], F32R)
            k_sb = pp.tile([128, 4, T], F32R)
            v_sb = pp.tile([128, KB_T, DQ], BF16)
            ones_bf = pp.tile([128, HD], BF16)
            bqkv_sb = pp.tile([128, 12], F32)
            bv_sb = pp.tile([128, DQ], F32)

            nc.vector.memset(ones_bf[:], 1.0)
            nc.sync.dma_start(out=bqkv_sb, in_=b_qkv.rearrange("(m p) -> p m", p=128))
            nc.sync.dma_start(out=bv_sb, in_=bcast_ap(b_qkv[2 * DQ:3 * DQ], 128))

            # ---------------- Phase A: QKV projections ----------------
            with (
                tc.tile_pool(name="phA", bufs=1) as pa,
                tc.tile_pool(name="wch", bufs=2) as wch,
                tc.tile_pool(name="psA", bufs=2, space="PSUM") as psa,
                tc.tile_pool(name="psV", bufs=2, space="PSUM") as psv,
            ):
                x_sb = pa.tile([128, KB_C, T], F32R)
                for kb in range(KB_C):
                    nc.sync.dma_start(
                        out=x_sb[:, kb, :],
                        in_=x_t[128 * kb:128 * (kb + 1), :].bitcast(F32R))

                # Q then K: out^T layout [dq, t], stationary w block, moving x^T
                for which, dst in ((0, q_sb), (1, k_sb)):
                    w_c = wch.tile([128, KB_C, DQ], F32R, tag="wchunk")
                    for kb in range(KB_C):
                        nc.sync.dma_start(
                            out=w_c[:, kb, :],
                            in_=w_qkv[128 * kb:128 * (kb + 1),
                                      DQ * which:DQ * (which + 1)].bitcast(F32R))
                    for m in range(4):
                        for nbh in range(2):
                            ps = psa.tile([128, 1024], F32)
                            for kb in range(KB_C):
                                for nb2 in range(2):
                                    tcol = 1024 * nbh + 512 * nb2
                                    nc.tensor.matmul(
                                        ps[:, 512 * nb2:512 * (nb2 + 1)],
                                        w_c[:, kb, 128 * m:128 * (m + 1)],
                                        x_sb[:, kb, tcol:tcol + 512],
                                        start=(kb == 0), stop=(kb == KB_C - 1))
                            nc.vector.tensor_scalar_add(
                                out=dst[:, m, 1024 * nbh:1024 * (nbh + 1)],
                                in0=ps[:],
                                scalar1=bqkv_sb[:, 4 * which + m:4 * which + m + 1])

                # V: natural [t, dv] layout; stationary x^T block, moving w_v
                w_c = wch.tile([128, KB_C, DQ], F32R, tag="wchunk")
                for kb in range(KB_C):
                    nc.sync.dma_start(
                        out=w_c[:, kb, :],
                        in_=w_qkv[128 * kb:128 * (kb + 1),
                                  2 * DQ:3 * DQ].bitcast(F32R))
                for tb in range(KB_T):
                    ps = psv.tile([128, DQ], F32)
                    for kb in range(KB_C):
                        nc.tensor.matmul(
                            ps[:],
                            x_sb[:, kb, 128 * tb:128 * (tb + 1)],
                            w_c[:, kb, :],
                            start=(kb == 0), stop=(kb == KB_C - 1))
                    nc.vector.tensor_add(v_sb[:, tb, :], ps[:], bv_sb[:])

            # ---------------- Phase B: attention + proj + RS ----------------
            with (
                tc.tile_pool(name="zb", bufs=1) as zb,
                tc.tile_pool(name="apool", bufs=3) as apool,
                tc.tile_pool(name="small", bufs=4) as small,
                tc.tile_pool(name="opool", bufs=3) as opool,
                tc.tile_pool(name="psS", bufs=3, space="PSUM") as pss,
                tc.tile_pool(name="psZ", bufs=2, space="PSUM") as psz,
            ):
                z_sb = zb.tile([128, 4, T], BF16)
                wp_sb = zb.tile([128, 4, DIM], BF16)
                bp_sb = zb.tile([128, DIM], F32)
                nc.sync.dma_start(
                    out=wp_sb, in_=w_proj.rearrange("(m p) c -> p m c", p=128))
                nc.sync.dma_start(out=bp_sb, in_=bcast_ap(b_proj[:], 128))

                def emit_proj_group(qb, tb4):
                    t0 = 512 * qb + 128 * tb4
                    for cb in range(2):
                        ppj = psz.tile([128, 512], F32, tag="z")
                        for m in range(4):
                            nc.tensor.matmul(
                                ppj[:],
                                z_sb[:, m, t0:t0 + 128],
                                wp_sb[:, m, 512 * cb:512 * (cb + 1)],
                                start=(m == 0), stop=(m == 3))
                        o = opool.tile([128, 512], F32, tag="o")
                        nc.vector.tensor_add(
                            o[:], ppj[:], bp_sb[:, 512 * cb:512 * (cb + 1)])
                        nc.sync.dma_start(
                            out=partial[t0:t0 + 128, 512 * cb:512 * (cb + 1)],
                            in_=o[:])

                def emit_rs(qb):
                    nc.gpsimd.collective_compute(
                        "ReduceScatter",
                        mybir.AluOpType.add,
                        ins=[partial[512 * qb:512 * (qb + 1), :]],
                        outs=[rs_out[256 * qb:256 * (qb + 1), :]],
                        replica_groups=groups,
                    )
                    nc.sync.dma_start(
                        out=out[256 * qb:256 * (qb + 1), :],
                        in_=rs_out[256 * qb:256 * (qb + 1), :])

                for qb in range(4):
                    q0 = 512 * qb
                    for hp in range(4):
                        z0 = psz.tile([128, 512], F32, tag="z")
                        z1 = psz.tile([128, 512], F32, tag="z")
                        for ch in range(KB_T // CH):
                            s0 = pss.tile([128, CH, 512], F32, tag="s")
                            s1 = pss.tile([128, CH, 512], F32, tag="s")
                            for i in range(CH):
                                kb = CH * ch + i
                                kc = 128 * kb
                                nc.tensor.matmul(
                                    s0[:, i, :],
                                    k_sb[0:64, hp, kc:kc + 128],
                                    q_sb[0:64, hp, q0:q0 + 512],
                                    start=True, stop=True)
                                nc.tensor.matmul(
                                    s1[:, i, :],
                                    k_sb[64:128, hp, kc:kc + 128],
                                    q_sb[64:128, hp, q0:q0 + 512],
                                    start=True, stop=True)
                            a0 = apool.tile([128, CH, 512], BF16, tag="a")
                            a1 = apool.tile([128, CH, 512], BF16, tag="a")
                            nc.scalar.activation(out=a0[:], in_=s0[:],
                                                 func=AF.Exp, scale=SCALE)
                            nc.scalar.activation(out=a1[:], in_=s1[:],
                                                 func=AF.Exp, scale=SCALE)
                            for i in range(CH):
                                kb = CH * ch + i
                                st = (kb == 0)
                                sp = (kb == KB_T - 1)
                                h0c = 64 * (2 * hp)
                                h1c = 64 * (2 * hp + 1)
                                nc.tensor.matmul(
                                    z0[0:64, :], v_sb[:, kb, h0c:h0c + 64],
                                    a0[:, i, :], start=st, stop=sp,
                                    tile_position=(0, 0))
                                nc.tensor.matmul(
                                    z0[64:128, :], ones_bf[:],
                                    a0[:, i, :], start=st, stop=sp,
                                    tile_position=(0, 64))
                                nc.tensor.matmul(
                                    z1[0:64, :], v_sb[:, kb, h1c:h1c + 64],
                                    a1[:, i, :], start=st, stop=sp,
                                    tile_position=(0, 0))
                                nc.tensor.matmul(
                                    z1[64:128, :], ones_bf[:],
                                    a1[:, i, :], start=st, stop=sp,
                                    tile_position=(0, 64))
                        # fast PSUM evacuation: one DVE copy frees the bank;
                        # reciprocal + normalize run off the critical path
                        for hh, zz in ((0, z0), (1, z1)):
                            zc = small.tile([128, 512], F32, tag="zc")
                            nc.vector.tensor_copy(zc[:], zz[:])
                            rinv = small.tile([64, 512], F32, tag="rinv")
                            nc.vector.reciprocal(rinv[:], zc[64:128, :])
                            nc.vector.tensor_mul(
                                z_sb[64 * hh:64 * hh + 64, hp, q0:q0 + 512],
                                zc[0:64, :], rinv[:])
                        # interleave previous q-block's output projection
                        if qb > 0:
                            emit_proj_group(qb - 1, hp)
                    if qb > 0:
                        emit_rs(qb - 1)
                for tb4 in range(4):
                    emit_proj_group(3, tb4)
                emit_rs(3)

    nc.compile()
    return nc


def _get_nc():
    if "nc" not in _CACHE:
        _CACHE["nc"] = _build()
    return _CACHE["nc"]


def kernel(x, w_qkv, b_qkv, w_proj, b_proj):
    from concourse.bass_utils import run_bass_kernel_spmd

    x = np.asarray(x, dtype=np.float32)
    w_qkv = np.asarray(w_qkv, dtype=np.float32)
    b_qkv = np.asarray(b_qkv, dtype=np.float32)
    w_proj = np.asarray(w_proj, dtype=np.float32)
    b_proj = np.asarray(b_proj, dtype=np.float32)

    nc = _get_nc()

    in_maps = []
    for c in range(N_CORES):
        b = c // 2
        hg = c % 2
        cols = slice(DQ * hg, DQ * (hg + 1))
        w_s = np.ascontiguousarray(np.concatenate(
            [w_qkv[:, 0:DIM][:, cols],
             w_qkv[:, DIM:2 * DIM][:, cols],
             w_qkv[:, 2 * DIM:3 * DIM][:, cols]], axis=1))
        b_s = np.ascontiguousarray(np.concatenate(
            [b_qkv[0:DIM][cols], b_qkv[DIM:2 * DIM][cols],
             b_qkv[2 * DIM:3 * DIM][cols]]))
        in_maps.append({
            "x_t": np.ascontiguousarray(x[b].T),
            "w_qkv_s": w_s,
            "b_qkv_s": b_s,
            "w_proj_s": np.ascontiguousarray(
                w_proj[DQ * hg:DQ * (hg + 1), :]).astype(ml_dtypes.bfloat16),
            "b_proj_h": (b_proj * 0.5).astype(np.float32),
        })

    res = run_bass_kernel_spmd(nc, in_maps, core_ids=list(range(N_CORES)))

    full = np.empty((B, T, DIM), dtype=np.float32)
    for c in range(N_CORES):
        b = c // 2
        p = c % 2
        o = res.results[c]["out"]
        for qb in range(4):
            full[b, 512 * qb + 256 * p:512 * qb + 256 * (p + 1), :] = \
                o[256 * qb:256 * (qb + 1), :]
    return full
